# revision 9
# baseline (speedup 1.0000x reference)
"""Multi-head attention forward on 8 Trainium2 NeuronCores (Bass/Tile).

Problem: B=2, S=2048, d_model=1024, 16 heads (depth 64), fp32.
  q/k/v = query @ W{q,k,v}; logits = q k^T / 8 + mask * -1e9;
  out = softmax(logits) v @ Wo.

Sharding (Megatron-style, hardcoded): core c handles batch b = c//4 and head
group hg = c%4 (4 heads = 256 of the 1024 head dims). Wq/Wk/Wv are
column-sharded, Wo row-sharded; each core emits a partial [S, 1024] output
(bf16) and the host sums the 4 partials per batch (the "all-reduce").

Per-core design (v2 — engine-balanced, all-bf16 datapath):
  * Everything on the PE is bf16 (lower power -> less HAM throttling, smaller
    LDWEIGHTS). Attention math runs transposed: qT/kT are [dh, S] so QK^T
    lands as logitsT [k, q] tiles straight off the PE.
  * The attention inner loop is software-pipelined: QK^T for tile kb+1 is
    issued before the exp/mask/AV chain of tile kb, so the PE never waits on
    ScalarE.
  * exp is split between ScalarE (EXP activation) and VectorE (Schraudolph
    bit-trick: u16 = logit*A + B, bits reinterpreted as bf16 ~= exp) to beat
    the single-engine softmax floor; GpSimd takes copies/broadcasts/norms.
  * Denominators come from a ones-column appended to V; reciprocal runs on
    DVE per head as a [1, 1024] op, broadcast via gpsimd partition_broadcast.
  * Output projection for q-chunk 0 is interleaved into chunk 1's attention;
    output is bf16 and the host does the final f32 partial-sum.
"""

import sys

import numpy as np

sys.path.insert(0, "/opt/trn_rl_repo")

B = 2
S = 2048
D = 1024
HEADS = 16
DEPTH = 64
CORES = 8
HG = 4          # head groups (cores per batch)
HPC = 4         # heads per core
DH = HPC * DEPTH  # per-core head width = 256

# Schraudolph exp in bf16 bits: u16 = round(logit * SCHR_A + SCHR_B)
# exp(0.125*l) = 2^(0.125*l*log2 e); bf16 bits = 128*(bexp+mant/128)
SCHR_A = 0.125 * 128.0 / float(np.log(2.0))
SCHR_B = 127.0 * 128.0 - 5.7 + 0.5

# kb tiles handled by the DVE bit-trick exp instead of ScalarE (per 16)
DVE_KBS = (5, 11)

_CACHE = {}


def _build_program():
    import concourse.bass as bass  # noqa: F401  (registers engines)
    import concourse.mybir as mybir
    import concourse.tile as tile
    from concourse import bacc
    from concourse.bass_interp import get_hw_module
    from concourse.masks import make_identity

    dt = mybir.dt
    f32, bf16, u16 = dt.float32, dt.bfloat16, dt.uint16
    MULT = mybir.AluOpType.mult
    ADD = mybir.AluOpType.add
    EXP = mybir.ActivationFunctionType.Exp

    nc = bacc.Bacc(
        "TRN2",
        target_bir_lowering=False,
        debug=False,
        enable_asserts=True,
        num_devices=CORES,
    )

    xT = nc.dram_tensor("xT", [D, S], bf16, kind="ExternalInput").ap()
    imaskT = nc.dram_tensor("imaskT", [S, S], bf16, kind="ExternalInput").ap()
    wq = nc.dram_tensor("wq", [D, DH], bf16, kind="ExternalInput").ap()
    wk = nc.dram_tensor("wk", [D, DH], bf16, kind="ExternalInput").ap()
    wv = nc.dram_tensor("wv", [D, DH], bf16, kind="ExternalInput").ap()
    wo = nc.dram_tensor("wo", [DH, D], bf16, kind="ExternalInput").ap()
    vones = nc.dram_tensor("vones", [128, HPC, 1], bf16, kind="ExternalInput").ap()
    out = nc.dram_tensor("out", [S, D], bf16, kind="ExternalOutput").ap()

    with tile.TileContext(nc) as tc:
        with tc.tile_pool(name="persist", bufs=1) as pp:
            qT = [pp.tile([128, S], bf16, tag=f"qT{g}", name=f"qT{g}") for g in range(2)]
            kT = [pp.tile([128, S], bf16, tag=f"kT{g}", name=f"kT{g}") for g in range(2)]
            vt = [pp.tile([128, HPC, DEPTH + 1], bf16, tag=f"v{i}", name=f"v{i}") for i in range(16)]
            wot = [pp.tile([128, D], bf16, tag=f"wo{g}", name=f"wo{g}") for g in range(2)]
            # per (qcp, g): attn rows for heads 2g, 2g+1 (normalized in place)
            ath = [[pp.tile([128, 1024], bf16, tag=f"ath{qc}{g}", name=f"ath{qc}{g}")
                    for g in range(2)] for qc in range(2)]
            mt = pp.tile([128, 16, S], bf16, tag="mask", name="mask")
            identb = pp.tile([128, 128], bf16, tag="identb", name="identb")
            dummy = pp.tile([1, 64], bf16, tag="dummy", name="dummy")

            # ---- DMA issue order: tiny, wk, xT, wq, wv, mask, wo ----
            nc.sync.dma_start(identb[0:1, 0:1], vones[0:1, 0:1, 0:1])  # warm queue
            wts = {}
            xtp = tc.tile_pool(name="xw", bufs=1)
            xw = xtp.__enter__()
            xt = [xw.tile([128, S], bf16, tag=f"x{d}", name=f"x{d}") for d in range(8)]
            for nm in ("wq", "wk", "wv"):
                wts[nm] = [xw.tile([128, DH], bf16, tag=f"{nm}{d}", name=f"{nm}{d}") for d in range(8)]
            for d in range(8):
                nc.sync.dma_start(wts["wk"][d][:], wk[d * 128:(d + 1) * 128, :])
            for d in range(8):
                nc.sync.dma_start(xt[d][:], xT[d * 128:(d + 1) * 128, :])
            for d in range(8):
                nc.sync.dma_start(wts["wq"][d][:], wq[d * 128:(d + 1) * 128, :])
            for d in range(8):
                nc.sync.dma_start(wts["wv"][d][:], wv[d * 128:(d + 1) * 128, :])
            imaskT_r = imaskT.rearrange("(t p) q -> p t q", p=128)
            for kb in range(16):
                nc.sync.dma_start(mt[:, kb:kb + 1, :], imaskT_r[:, kb:kb + 1, :])
            for g in range(2):
                nc.sync.dma_start(wot[g][:], wo[g * 128:(g + 1) * 128, :])
            for st in range(16):
                nc.sync.dma_start(vt[st][:, :, DEPTH:DEPTH + 1], vones[:])

            # identity (bf16) + exp-table preload + PE warm spin
            make_identity(nc, identb[:])
            nc.scalar.activation(dummy[:], identb[0:1, 0:64], EXP, scale=1.0)
            with tc.tile_pool(name="psW", bufs=2, space="PSUM") as psW:
                for w in range(64):
                    psw = psW.tile([128, 128], f32, tag="warm", name="warm")
                    nc.tensor.matmul(psw[:], identb[:], identb[:],
                                     start=True, stop=True)

            # ---- projections (k, then q chunks sc0/sc1, then v) ----
            psA_cm = tc.tile_pool(name="psA", bufs=4, space="PSUM")
            psA = psA_cm.__enter__()

            def proj_qk(wt, dst, g, sc, pool, tag="proj"):
                ps = pool.tile([128, 512], f32, tag=tag, name=tag)
                for d in range(8):
                    nc.tensor.matmul(
                        ps[:],
                        wt[d][:, g * 128:(g + 1) * 128],
                        xt[d][:, sc * 512:(sc + 1) * 512],
                        start=(d == 0), stop=(d == 7),
                    )
                nc.vector.tensor_copy(dst[g][:, sc * 512:(sc + 1) * 512], ps[:])

            for g in range(2):
                for sc in range(4):
                    proj_qk(wts["wk"], kT, g, sc, psA)
            for g in range(2):
                for sc in range(2):
                    proj_qk(wts["wq"], qT, g, sc, psA)
            for st in range(16):
                ps = psA.tile([128, DH], f32, tag="proj", name="proj")
                for d in range(8):
                    nc.tensor.matmul(
                        ps[:],
                        xt[d][:, st * 128:(st + 1) * 128],
                        wts["wv"][d][:],
                        start=(d == 0), stop=(d == 7),
                    )
                nc.vector.tensor_copy(
                    vt[st][:, :, 0:DEPTH],
                    ps[:].rearrange("p (h e) -> p h e", h=HPC),
                )
            psA_cm.__exit__(None, None, None)

            # ---- attention ----
            with tc.tile_pool(name="attn", bufs=1) as ab, \
                 tc.tile_pool(name="exs", bufs=3) as exs, \
                 tc.tile_pool(name="psL", bufs=2, space="PSUM") as psL, \
                 tc.tile_pool(name="psO", bufs=1, space="PSUM") as psO, \
                 tc.tile_pool(name="psB", bufs=2, space="PSUM") as psB:

                def emit_qk(qcp, h, kb):
                    g, po = h // 2, (h % 2) * 64
                    psl = psL.tile([128, 1024], f32, tag="lg", name="lg")
                    for half in range(2):
                        hs = slice(half * 512, (half + 1) * 512)
                        qh = slice(qcp * 1024 + half * 512,
                                   qcp * 1024 + half * 512 + 512)
                        nc.tensor.matmul(
                            psl[:, hs],
                            kT[g][po:po + 64, kb * 128:(kb + 1) * 128],
                            qT[g][po:po + 64, qh],
                            start=True, stop=True,
                        )
                    return psl

                # deferred PE side-tasks, popped one per inner unit
                pe_tasks = []

                def emit_q23():
                    for g in range(2):
                        for sc in range(2, 4):
                            pe_tasks.append(
                                lambda g=g, sc=sc: proj_qk(
                                    wts["wq"], qT, g, sc, psB, tag="po"))

                def emit_outproj(qcp):
                    # 8 st tiles x (2 nch psum [128,512] x 2 g) + copies + DMA
                    def one_st(st):
                        ot = ab.tile([128, D], bf16, tag="ot", name="ot", bufs=2)
                        for nch in range(2):
                            psf = psB.tile([128, 512], f32, tag="po", name="po")
                            for g in range(2):
                                nc.tensor.matmul(
                                    psf[:],
                                    ath[qcp][g][:, st * 128 - qcp * 1024:
                                                st * 128 - qcp * 1024 + 128],
                                    wot[g][:, nch * 512:(nch + 1) * 512],
                                    start=(g == 0), stop=(g == 1),
                                )
                            if nch == 0:
                                nc.scalar.copy(
                                    ot[:, nch * 512:(nch + 1) * 512], psf[:])
                            else:
                                nc.vector.tensor_copy(
                                    ot[:, nch * 512:(nch + 1) * 512], psf[:])
                        nc.sync.dma_start(out[st * 128:(st + 1) * 128, :], ot[:])
                    for st in range(qcp * 8, qcp * 8 + 8):
                        pe_tasks.append(lambda st=st: one_st(st))

                def head_epilogue(qcp, h, dden):
                    # 1/den -> bf16 -> broadcast -> scale attn rows in place
                    g, po = h // 2, (h % 2) * 64
                    rden = ab.tile([1, 1024], f32, tag="rden", name="rden", bufs=2)
                    nc.vector.reciprocal(rden[:], dden[:])
                    rdb = ab.tile([1, 1024], bf16, tag="rdb", name="rdb", bufs=2)
                    nc.gpsimd.tensor_copy(rdb[:], rden[:])
                    rbc = ab.tile([128, 1024], bf16, tag="rbc", name="rbc", bufs=2)
                    nc.gpsimd.partition_broadcast(rbc[:], rdb[:])
                    nc.vector.tensor_tensor(
                        ath[qcp][g][po:po + 64, :],
                        ath[qcp][g][po:po + 64, :], rbc[po:po + 64, :], MULT)

                for qcp in range(2):
                    if qcp == 0:
                        emit_q23()
                    else:
                        emit_outproj(0)
                    units = [(h, kb) for h in range(HPC) for kb in range(16)]
                    psl_next = emit_qk(qcp, 0, 0)
                    pend_epi = None
                    for i, (h, kb) in enumerate(units):
                        g, po = h // 2, (h % 2) * 64
                        psl = psl_next
                        if i + 1 < len(units):
                            hn, kbn = units[i + 1]
                            psl_next = emit_qk(qcp, hn, kbn)
                        ex = exs.tile([128, 1024], bf16, tag="ex", name="ex", bufs=4)
                        if (kb % 16) in DVE_KBS:
                            nc.vector.tensor_scalar(
                                ex[:].bitcast(u16), psl[:],
                                SCHR_A, SCHR_B, MULT, ADD)
                        else:
                            nc.scalar.activation(ex[:], psl[:], EXP, scale=0.125)
                        em = exs.tile([128, 1024], bf16, tag="em", name="em", bufs=4)
                        nc.vector.tensor_tensor(
                            em[:], ex[:], mt[:, kb, qcp * 1024:(qcp + 1) * 1024],
                            MULT)
                        for half in range(2):
                            hs = slice(half * 512, (half + 1) * 512)
                            pso = psO.tile([65, 512], f32, tag=f"av{half}",
                                           name=f"av{half}")
                            nc.tensor.matmul(
                                pso[:], vt[kb][:, h, :], em[:, hs],
                                start=(kb == 0), stop=(kb == 15),
                            )
                            if kb == 15:
                                qs = slice(half * 512, half * 512 + 512)
                                nc.vector.tensor_copy(
                                    ath[qcp][g][po:po + 64, qs], pso[0:64, :])
                                if half == 0:
                                    dden = ab.tile([1, 1024], f32, tag="dden",
                                                   name="dden", bufs=2)
                                nc.scalar.copy(dden[0:1, qs], pso[64:65, :])
                        if kb == 15:
                            pend_epi = (qcp, h, dden)
                        if pe_tasks and (i % 2 == 1):
                            pe_tasks.pop(0)()
                        if pend_epi is not None and kb == 2:
                            head_epilogue(*pend_epi)
                            pend_epi = None
                    if pend_epi is not None:
                        head_epilogue(*pend_epi)
                        pend_epi = None
                    while qcp == 0 and pe_tasks:
                        pe_tasks.pop(0)()
                emit_outproj(1)
                while pe_tasks:
                    pe_tasks.pop(0)()

            xtp.__exit__(None, None, None)

    nc.compile()
    nc.m = get_hw_module(nc.m)
    return nc


def _get_program():
    if "nc" not in _CACHE:
        _CACHE["nc"] = _build_program()
    return _CACHE["nc"]


def _make_in_maps(query, attention_mask, Wq, Wk, Wv, Wo):
    import ml_dtypes

    bf = ml_dtypes.bfloat16
    in_maps = []
    imaskT_b = []
    xT_b = []
    for b in range(B):
        imaskT_b.append(
            np.ascontiguousarray(1 - attention_mask[b, 0].T).astype(bf)
        )
        xT_b.append(np.ascontiguousarray(query[b].T).astype(bf))
    for c in range(CORES):
        b, hg = c // HG, c % HG
        cs = slice(hg * DH, (hg + 1) * DH)
        in_maps.append({
            "xT": xT_b[b],
            "imaskT": imaskT_b[b],
            "wq": np.ascontiguousarray(Wq[:, cs]).astype(bf),
            "wk": np.ascontiguousarray(Wk[:, cs]).astype(bf),
            "wv": np.ascontiguousarray(Wv[:, cs]).astype(bf),
            "wo": np.ascontiguousarray(Wo[cs, :]).astype(bf),
            "vones": np.ones((128, HPC, 1), dtype=bf),
        })
    return in_maps


def _run(inputs, trace=False):
    from concourse.bass_utils import run_bass_kernel_spmd

    nc = _get_program()
    in_maps = _make_in_maps(**inputs)
    res = run_bass_kernel_spmd(
        nc, in_maps, core_ids=list(range(CORES)), trace=trace,
    )
    outs = [res.results[c]["out"].astype(np.float32) for c in range(CORES)]
    full = np.empty((B, S, D), dtype=np.float32)
    for b in range(B):
        acc = outs[4 * b]
        for hg in range(1, HG):
            acc = acc + outs[4 * b + hg]
        full[b] = acc
    return full, res


def kernel(query, attention_mask, Wq, Wk, Wv, Wo):
    full, _ = _run(dict(
        query=np.asarray(query), attention_mask=np.asarray(attention_mask),
        Wq=np.asarray(Wq), Wk=np.asarray(Wk), Wv=np.asarray(Wv),
        Wo=np.asarray(Wo),
    ))
    return full


# revision 20
# speedup vs baseline: 1.1942x; 1.1942x over previous
"""Multi-head attention forward on 8 Trainium2 NeuronCores (Bass/Tile).

Problem: B=2, S=2048, d_model=1024, 16 heads (depth 64), fp32.
  q/k/v = query @ W{q,k,v}; logits = q k^T / 8 + mask * -1e9;
  out = softmax(logits) v @ Wo.

Sharding (Megatron-style, hardcoded): core c handles batch b = c//4 and head
group hg = c%4 (4 heads = 256 of the 1024 head dims). Wq/Wk/Wv are
column-sharded, Wo row-sharded; each core emits a partial [S, 1024] output
(bf16) and the host sums the 4 partials per batch (the "all-reduce").

Per-core design (v2 — engine-balanced, all-bf16 datapath):
  * Everything on the PE is bf16 (lower power -> less HAM throttling, smaller
    LDWEIGHTS). Attention math runs transposed: qT/kT are [dh, S] so QK^T
    lands as logitsT [k, q] tiles straight off the PE.
  * The attention inner loop is software-pipelined: QK^T for tile kb+1 is
    issued before the exp/mask/AV chain of tile kb, so the PE never waits on
    ScalarE.
  * exp is split between ScalarE (EXP activation) and VectorE (Schraudolph
    bit-trick: u16 = logit*A + B, bits reinterpreted as bf16 ~= exp) to beat
    the single-engine softmax floor; GpSimd takes copies/broadcasts/norms.
  * Denominators come from a ones-column appended to V; reciprocal runs on
    DVE per head as a [1, 1024] op, broadcast via gpsimd partition_broadcast.
  * Output projection for q-chunk 0 is interleaved into chunk 1's attention;
    output is bf16 and the host does the final f32 partial-sum.
"""

import sys

import numpy as np

sys.path.insert(0, "/opt/trn_rl_repo")

B = 2
S = 2048
D = 1024
HEADS = 16
DEPTH = 64
CORES = 8
HG = 4          # head groups (cores per batch)
HPC = 4         # heads per core
DH = HPC * DEPTH  # per-core head width = 256

# Schraudolph exp in bf16 bits: u16 = round(logit * SCHR_A + SCHR_B)
# exp(0.125*l) = 2^(0.125*l*log2 e); bf16 bits = 128*(bexp+mant/128)
SCHR_A = 0.125 * 128.0 / float(np.log(2.0))
SCHR_B = 127.0 * 128.0 - 5.7 + 0.5

# kb tiles handled by the DVE bit-trick exp instead of ScalarE (per 16)
DVE_KBS = (5, 11)

_CACHE = {}


def _build_program():
    import concourse.bass as bass  # noqa: F401  (registers engines)
    import concourse.mybir as mybir
    import concourse.tile as tile
    from concourse import bacc
    from concourse.bass_interp import get_hw_module
    from concourse.masks import make_identity

    dt = mybir.dt
    f32, bf16, u16 = dt.float32, dt.bfloat16, dt.uint16
    MULT = mybir.AluOpType.mult
    ADD = mybir.AluOpType.add
    EXP = mybir.ActivationFunctionType.Exp

    nc = bacc.Bacc(
        "TRN2",
        target_bir_lowering=False,
        debug=False,
        enable_asserts=True,
        num_devices=CORES,
    )

    f32r = dt.float32r
    xT = nc.dram_tensor("xT", [D, S], bf16, kind="ExternalInput").ap()
    imaskT = nc.dram_tensor("imaskT", [S, S], bf16, kind="ExternalInput").ap()
    wq = nc.dram_tensor("wq", [D, DH], bf16, kind="ExternalInput").ap()
    wk = nc.dram_tensor("wk", [D, DH], bf16, kind="ExternalInput").ap()
    wv = nc.dram_tensor("wv", [D, DH], bf16, kind="ExternalInput").ap()
    wo = nc.dram_tensor("wo", [DH, D], bf16, kind="ExternalInput").ap()
    vones = nc.dram_tensor("vones", [128, HPC, 1], bf16, kind="ExternalInput").ap()
    ones_rd = nc.dram_tensor("ones_rd", [1, DEPTH], bf16, kind="ExternalInput").ap()
    out = nc.dram_tensor("out", [S, D], bf16, kind="ExternalOutput").ap()

    with tile.TileContext(nc) as tc:
        with tc.tile_pool(name="persist", bufs=1) as pp:
            qT = [pp.tile([128, S], bf16, tag=f"qT{g}", name=f"qT{g}") for g in range(2)]
            kT = [pp.tile([128, S], bf16, tag=f"kT{g}", name=f"kT{g}") for g in range(2)]
            vt = [pp.tile([128, HPC, DEPTH + 1], bf16, tag=f"v{i}", name=f"v{i}") for i in range(16)]
            wot = [pp.tile([128, D], bf16, tag=f"wo{g}", name=f"wo{g}") for g in range(2)]
            # per (qcp, g): attn rows for heads 2g, 2g+1 (normalized in place)
            ath = [[pp.tile([128, 1024], bf16, tag=f"ath{qc}{g}", name=f"ath{qc}{g}")
                    for g in range(2)] for qc in range(2)]
            mt = pp.tile([128, 16, S], bf16, tag="mask", name="mask")
            identb = pp.tile([128, 128], bf16, tag="identb", name="identb")
            dummy = pp.tile([1, 64], bf16, tag="dummy", name="dummy")
            ones_r = pp.tile([1, DEPTH], bf16, tag="ones_r", name="ones_r")
            # per-qcp denominator staging: row-major [1,1024] per head,
            # partition-major [128, 4h x 8c] for the reciprocal, and the
            # broadcast row [1, 4096] feeding the rank-1 psc matmuls.
            dden = [pp.tile([1, HPC, 1024], bf16, tag=f"dd{qc}", name=f"dd{qc}")
                    for qc in range(2)]
            den_pm = [pp.tile([128, HPC, 8], bf16, tag=f"dpm{qc}", name=f"dpm{qc}")
                      for qc in range(2)]
            rden_pm = [pp.tile([128, HPC, 8], bf16, tag=f"rpm{qc}", name=f"rpm{qc}")
                       for qc in range(2)]
            rrow = [pp.tile([1, 4096], bf16, tag=f"rr{qc}", name=f"rr{qc}")
                    for qc in range(2)]
            dsc_cm = tc.tile_pool(name="dscr", bufs=1, space="DRAM")
            dsc = dsc_cm.__enter__()
            den_scr = [dsc.tile([HPC, 8, 128], bf16, tag=f"ds{qc}", name=f"ds{qc}")
                       for qc in range(2)]
            rden_scr = [dsc.tile([HPC, 8, 128], bf16, tag=f"rs{qc}", name=f"rs{qc}")
                        for qc in range(2)]

            # ---- DMA issue order: tiny, wk, xT, wq, wv, mask, wo ----
            nc.sync.dma_start(ones_r[:], ones_rd[:])
            ab_cm = tc.tile_pool(name="attn", bufs=1)
            ab = ab_cm.__enter__()
            exs_cm = tc.tile_pool(name="exs", bufs=3)
            exs = exs_cm.__enter__()
            wts = {}
            xtp = tc.tile_pool(name="xw", bufs=1)
            xw = xtp.__enter__()
            xt = [xw.tile([128, S], bf16, tag=f"x{d}", name=f"x{d}") for d in range(8)]
            for nm in ("wq", "wk", "wv"):
                wts[nm] = [xw.tile([128, DH], bf16, tag=f"{nm}{d}", name=f"{nm}{d}") for d in range(8)]
            for d in range(8):
                nc.sync.dma_start(wts["wk"][d][:], wk[d * 128:(d + 1) * 128, :])
            for d in range(8):
                nc.sync.dma_start(xt[d][:], xT[d * 128:(d + 1) * 128, :])
            for d in range(8):
                nc.sync.dma_start(wts["wq"][d][:], wq[d * 128:(d + 1) * 128, :])
            for d in range(8):
                nc.sync.dma_start(wts["wv"][d][:], wv[d * 128:(d + 1) * 128, :])
            imaskT_r = imaskT.rearrange("(t p) q -> p t q", p=128)
            for kb in range(16):
                nc.sync.dma_start(mt[:, kb:kb + 1, :], imaskT_r[:, kb:kb + 1, :])
            for g in range(2):
                nc.sync.dma_start(wot[g][:], wo[g * 128:(g + 1) * 128, :])
            for st in range(16):
                nc.sync.dma_start(vt[st][:, :, DEPTH:DEPTH + 1], vones[:])

            # identity (bf16) + exp-table preload + PE warm spin
            make_identity(nc, identb[:])
            nc.scalar.activation(dummy[:], identb[0:1, 0:64], EXP, scale=1.0)
            with tc.tile_pool(name="psW", bufs=2, space="PSUM") as psW:
                for w in range(64):
                    psw = psW.tile([128, 128], f32, tag="warm", name="warm")
                    nc.tensor.matmul(psw[:], identb[:], identb[:],
                                     start=True, stop=True)

            # ---- projections (k, then q chunks sc0/sc1, then v) ----
            psA_cm = tc.tile_pool(name="psA", bufs=4, space="PSUM")
            psA = psA_cm.__enter__()

            def proj_qk(wt, dst, g, sc, pool, tag="proj"):
                ps = pool.tile([128, 512], f32, tag=tag, name=tag)
                for d in range(8):
                    nc.tensor.matmul(
                        ps[:],
                        wt[d][:, g * 128:(g + 1) * 128],
                        xt[d][:, sc * 512:(sc + 1) * 512],
                        start=(d == 0), stop=(d == 7),
                    )
                nc.vector.tensor_copy(dst[g][:, sc * 512:(sc + 1) * 512], ps[:])

            for g in range(2):
                for sc in range(4):
                    proj_qk(wts["wk"], kT, g, sc, psA)
            for g in range(2):
                for sc in range(2):
                    proj_qk(wts["wq"], qT, g, sc, psA)
            for st in range(16):
                ps = psA.tile([128, DH], f32, tag="proj", name="proj")
                for d in range(8):
                    nc.tensor.matmul(
                        ps[:],
                        xt[d][:, st * 128:(st + 1) * 128],
                        wts["wv"][d][:],
                        start=(d == 0), stop=(d == 7),
                    )
                nc.vector.tensor_copy(
                    vt[st][:, :, 0:DEPTH],
                    ps[:].rearrange("p (h e) -> p h e", h=HPC),
                )
            psA_cm.__exit__(None, None, None)

            # ---- attention ----
            with tc.tile_pool(name="psL", bufs=2, space="PSUM") as psL, \
                 tc.tile_pool(name="psO", bufs=1, space="PSUM") as psO, \
                 tc.tile_pool(name="psB", bufs=2, space="PSUM") as psB:

                def emit_qk(qcp, h, kb):
                    g, po = h // 2, (h % 2) * 64
                    psl = psL.tile([128, 1024], f32, tag="lg", name="lg")
                    for half in range(2):
                        hs = slice(half * 512, (half + 1) * 512)
                        qh = slice(qcp * 1024 + half * 512,
                                   qcp * 1024 + half * 512 + 512)
                        nc.tensor.matmul(
                            psl[:, hs],
                            kT[g][po:po + 64, kb * 128:(kb + 1) * 128],
                            qT[g][po:po + 64, qh],
                            start=True, stop=True,
                        )
                    return psl

                # deferred PE side-tasks, popped one per inner unit
                pe_tasks = []

                def emit_q23():
                    for g in range(2):
                        for sc in range(2, 4):
                            pe_tasks.append(
                                lambda g=g, sc=sc: proj_qk(
                                    wts["wq"], qT, g, sc, psB, tag="po"))

                def emit_outproj(qcp):
                    # 8 st tiles x (2 nch psum [128,512] x 2 g) + copies + DMA
                    def one_st(st):
                        ot = ab.tile([128, D], bf16, tag="ot", name="ot", bufs=2)
                        for nch in range(2):
                            psf = psB.tile([128, 512], f32, tag="po", name="po")
                            for g in range(2):
                                nc.tensor.matmul(
                                    psf[:],
                                    ath[qcp][g][:, st * 128 - qcp * 1024:
                                                st * 128 - qcp * 1024 + 128],
                                    wot[g][:, nch * 512:(nch + 1) * 512],
                                    start=(g == 0), stop=(g == 1),
                                )
                            if nch == 0:
                                nc.scalar.copy(
                                    ot[:, nch * 512:(nch + 1) * 512], psf[:])
                            else:
                                nc.vector.tensor_copy(
                                    ot[:, nch * 512:(nch + 1) * 512], psf[:])
                        nc.sync.dma_start(out[st * 128:(st + 1) * 128, :], ot[:])
                    for st in range(qcp * 8, qcp * 8 + 8):
                        pe_tasks.append(lambda st=st: one_st(st))

                def head_den_dma(qcp, h):
                    # park this head's denominator row in DRAM scratch
                    nc.sync.dma_start(den_scr[qcp][h, :, :],
                                      dden[qcp][0:1, h, :])

                def qcp_epilogue(qcp):
                    # denominators partition-major via DRAM round-trip, one
                    # [128,32] reciprocal, back to a row, rank-1 + norm.
                    nc.sync.dma_start(den_pm[qcp][:],
                                      den_scr[qcp].rearrange("h c p -> p h c"))
                    with nc.allow_low_precision(reason="bf16 denominators"):
                        nc.vector.reciprocal(rden_pm[qcp][:], den_pm[qcp][:])
                    nc.sync.dma_start(rden_scr[qcp].rearrange("h c p -> p h c"),
                                      rden_pm[qcp][:])
                    nc.sync.dma_start(rrow[qcp][0:1, :], rden_scr[qcp][:, :, :])
                    for g in range(2):
                        for half in range(2):
                            hs = slice(half * 512, (half + 1) * 512)
                            psc = psB.tile([128, 512], f32, tag="po", name="psc")
                            for hh in range(2):
                                h = 2 * g + hh
                                nc.tensor.matmul(
                                    psc[hh * 64:(hh + 1) * 64, :],
                                    ones_r[:],
                                    rrow[qcp][0:1, h * 1024 + half * 512:
                                              h * 1024 + half * 512 + 512],
                                    start=True, stop=True,
                                )
                            nc.vector.tensor_tensor(
                                ath[qcp][g][:, hs],
                                ath[qcp][g][:, hs], psc[:], MULT)

                for qcp in range(2):
                    if qcp == 0:
                        emit_q23()
                        pop_every = 4
                    else:
                        xtp.__exit__(None, None, None)
                        pe_tasks.append(lambda: qcp_epilogue(0))
                        emit_outproj(0)
                        pop_every = 2
                    units = [(h, kb) for h in range(HPC) for kb in range(16)]
                    psl_next = emit_qk(qcp, 0, 0)
                    for i, (h, kb) in enumerate(units):
                        g, po = h // 2, (h % 2) * 64
                        psl = psl_next
                        if i + 1 < len(units):
                            hn, kbn = units[i + 1]
                            psl_next = emit_qk(qcp, hn, kbn)
                        ex = exs.tile([128, 1024], bf16, tag="ex", name="ex", bufs=4)
                        if (kb % 16) in DVE_KBS:
                            nc.vector.tensor_scalar(
                                ex[:].bitcast(u16), psl[:],
                                SCHR_A, SCHR_B, MULT, ADD)
                        else:
                            nc.scalar.activation(ex[:], psl[:], EXP, scale=0.125)
                        em = exs.tile([128, 1024], bf16, tag="em", name="em", bufs=4)
                        nc.vector.tensor_tensor(
                            em[:], ex[:], mt[:, kb, qcp * 1024:(qcp + 1) * 1024],
                            MULT)
                        for half in range(2):
                            hs = slice(half * 512, (half + 1) * 512)
                            pso = psO.tile([65, 512], f32, tag=f"av{half}",
                                           name=f"av{half}")
                            nc.tensor.matmul(
                                pso[:], vt[kb][:, h, :], em[:, hs],
                                start=(kb == 0), stop=(kb == 15),
                            )
                            if kb == 15:
                                nc.vector.tensor_copy(
                                    ath[qcp][g][po:po + 64, hs], pso[0:64, :])
                                nc.scalar.copy(
                                    dden[qcp][0:1, h, hs], pso[64:65, :])
                        if kb == 15:
                            head_den_dma(qcp, h)
                        if pe_tasks and (i % pop_every == 1):
                            pe_tasks.pop(0)()
                    while qcp == 0 and pe_tasks:
                        pe_tasks.pop(0)()
                qcp_epilogue(1)
                emit_outproj(1)
                while pe_tasks:
                    pe_tasks.pop(0)()
            exs_cm.__exit__(None, None, None)
            ab_cm.__exit__(None, None, None)
            dsc_cm.__exit__(None, None, None)

    nc.compile()
    nc.m = get_hw_module(nc.m)
    return nc


def _get_program():
    if "nc" not in _CACHE:
        _CACHE["nc"] = _build_program()
    return _CACHE["nc"]


def _make_in_maps(query, attention_mask, Wq, Wk, Wv, Wo):
    import ml_dtypes

    bf = ml_dtypes.bfloat16
    in_maps = []
    imaskT_b = []
    xT_b = []
    for b in range(B):
        imaskT_b.append(
            np.ascontiguousarray(1 - attention_mask[b, 0].T).astype(bf)
        )
        xT_b.append(np.ascontiguousarray(query[b].T).astype(bf))
    for c in range(CORES):
        b, hg = c // HG, c % HG
        cs = slice(hg * DH, (hg + 1) * DH)
        in_maps.append({
            "xT": xT_b[b],
            "imaskT": imaskT_b[b],
            "wq": np.ascontiguousarray(Wq[:, cs]).astype(bf),
            "wk": np.ascontiguousarray(Wk[:, cs]).astype(bf),
            "wv": np.ascontiguousarray(Wv[:, cs]).astype(bf),
            "wo": np.ascontiguousarray(Wo[cs, :]).astype(bf),
            "vones": np.ones((128, HPC, 1), dtype=bf),
            "ones_rd": np.ones((1, DEPTH), dtype=bf),
        })
    return in_maps


def _run(inputs, trace=False):
    from concourse.bass_utils import run_bass_kernel_spmd

    nc = _get_program()
    in_maps = _make_in_maps(**inputs)
    res = run_bass_kernel_spmd(
        nc, in_maps, core_ids=list(range(CORES)), trace=trace,
    )
    outs = [res.results[c]["out"].astype(np.float32) for c in range(CORES)]
    full = np.empty((B, S, D), dtype=np.float32)
    for b in range(B):
        acc = outs[4 * b]
        for hg in range(1, HG):
            acc = acc + outs[4 * b + hg]
        full[b] = acc
    return full, res


def kernel(query, attention_mask, Wq, Wk, Wv, Wo):
    full, _ = _run(dict(
        query=np.asarray(query), attention_mask=np.asarray(attention_mask),
        Wq=np.asarray(Wq), Wk=np.asarray(Wk), Wv=np.asarray(Wv),
        Wo=np.asarray(Wo),
    ))
    return full


# revision 22
# speedup vs baseline: 1.2771x; 1.0694x over previous
"""Multi-head attention forward on 8 Trainium2 NeuronCores (Bass/Tile).

Problem: B=2, S=2048, d_model=1024, 16 heads (depth 64), fp32.
  q/k/v = query @ W{q,k,v}; logits = q k^T / 8 + mask * -1e9;
  out = softmax(logits) v @ Wo.

Sharding (Megatron-style, hardcoded): core c handles batch b = c//4 and head
group hg = c%4 (4 heads = 256 of the 1024 head dims). Wq/Wk/Wv are
column-sharded, Wo row-sharded; each core emits a partial [S, 1024] output
(bf16) and the host sums the 4 partials per batch (the "all-reduce").

Per-core design (v2 — engine-balanced, all-bf16 datapath):
  * Everything on the PE is bf16 (lower power -> less HAM throttling, smaller
    LDWEIGHTS). Attention math runs transposed: qT/kT are [dh, S] so QK^T
    lands as logitsT [k, q] tiles straight off the PE.
  * The attention inner loop is software-pipelined: QK^T for tile kb+1 is
    issued before the exp/mask/AV chain of tile kb, so the PE never waits on
    ScalarE.
  * exp is split between ScalarE (EXP activation) and VectorE (Schraudolph
    bit-trick: u16 = logit*A + B, bits reinterpreted as bf16 ~= exp) to beat
    the single-engine softmax floor; GpSimd takes copies/broadcasts/norms.
  * Denominators come from a ones-column appended to V; reciprocal runs on
    DVE per head as a [1, 1024] op, broadcast via gpsimd partition_broadcast.
  * Output projection for q-chunk 0 is interleaved into chunk 1's attention;
    output is bf16 and the host does the final f32 partial-sum.
"""

import sys

import numpy as np

sys.path.insert(0, "/opt/trn_rl_repo")

B = 2
S = 2048
D = 1024
HEADS = 16
DEPTH = 64
CORES = 8
HG = 4          # head groups (cores per batch)
HPC = 4         # heads per core
DH = HPC * DEPTH  # per-core head width = 256

# Schraudolph exp in bf16 bits: u16 = round(logit * SCHR_A + SCHR_B)
# exp(0.125*l) = 2^(0.125*l*log2 e); bf16 bits = 128*(bexp+mant/128)
SCHR_A = 0.125 * 128.0 / float(np.log(2.0))
SCHR_B = 127.0 * 128.0 - 5.7 + 0.5

# kb tiles handled by the DVE bit-trick exp instead of ScalarE (per 16)
DVE_KBS = (5, 11)

_CACHE = {}


def _build_program():
    import concourse.bass as bass  # noqa: F401  (registers engines)
    import concourse.mybir as mybir
    import concourse.tile as tile
    from concourse import bacc
    from concourse.bass_interp import get_hw_module
    from concourse.masks import make_identity

    dt = mybir.dt
    f32, bf16, u16 = dt.float32, dt.bfloat16, dt.uint16
    MULT = mybir.AluOpType.mult
    ADD = mybir.AluOpType.add
    EXP = mybir.ActivationFunctionType.Exp

    nc = bacc.Bacc(
        "TRN2",
        target_bir_lowering=False,
        debug=False,
        enable_asserts=True,
        num_devices=CORES,
    )

    f32r = dt.float32r
    xT = nc.dram_tensor("xT", [D, S], bf16, kind="ExternalInput").ap()
    imaskT = nc.dram_tensor("imaskT", [S, S], bf16, kind="ExternalInput").ap()
    wq = nc.dram_tensor("wq", [D, DH], bf16, kind="ExternalInput").ap()
    wk = nc.dram_tensor("wk", [D, DH], bf16, kind="ExternalInput").ap()
    wv = nc.dram_tensor("wv", [D, DH], bf16, kind="ExternalInput").ap()
    wo = nc.dram_tensor("wo", [DH, D], bf16, kind="ExternalInput").ap()
    vones = nc.dram_tensor("vones", [128, HPC, 1], bf16, kind="ExternalInput").ap()
    ones_rd = nc.dram_tensor("ones_rd", [1, DEPTH], bf16, kind="ExternalInput").ap()
    out = nc.dram_tensor("out", [S, D], bf16, kind="ExternalOutput").ap()

    with tile.TileContext(nc) as tc:
        with tc.tile_pool(name="persist", bufs=1) as pp:
            qT = [pp.tile([128, S], bf16, tag=f"qT{g}", name=f"qT{g}") for g in range(2)]
            kT = [pp.tile([128, S], bf16, tag=f"kT{g}", name=f"kT{g}") for g in range(2)]
            vt = [pp.tile([128, HPC, DEPTH + 1], bf16, tag=f"v{i}", name=f"v{i}") for i in range(16)]
            wot = [pp.tile([128, D], bf16, tag=f"wo{g}", name=f"wo{g}") for g in range(2)]
            # per (qcp, g): attn rows for heads 2g, 2g+1 (normalized in place)
            ath = [[pp.tile([128, 1024], bf16, tag=f"ath{qc}{g}", name=f"ath{qc}{g}")
                    for g in range(2)] for qc in range(2)]
            mt = pp.tile([128, 16, S], bf16, tag="mask", name="mask")
            identb = pp.tile([128, 128], bf16, tag="identb", name="identb")
            dummy = pp.tile([1, 64], bf16, tag="dummy", name="dummy")
            ones_r = pp.tile([1, DEPTH], bf16, tag="ones_r", name="ones_r")
            # per-qcp denominator staging: row-major [1,1024] per head,
            # partition-major [128, 4h x 8c] for the reciprocal, and the
            # broadcast row [1, 4096] feeding the rank-1 psc matmuls.
            dden = [pp.tile([1, HPC, 1024], bf16, tag=f"dd{qc}", name=f"dd{qc}")
                    for qc in range(2)]
            den_pm = [pp.tile([128, 32], bf16, tag=f"dpm{qc}", name=f"dpm{qc}")
                      for qc in range(2)]
            rden_pm = [pp.tile([128, 32], bf16, tag=f"rpm{qc}", name=f"rpm{qc}")
                       for qc in range(2)]
            rrow = [pp.tile([1, 4096], bf16, tag=f"rr{qc}", name=f"rr{qc}")
                    for qc in range(2)]
            dsc_cm = tc.tile_pool(name="dscr", bufs=1, space="DRAM")
            dsc = dsc_cm.__enter__()
            den_scr = [dsc.tile([32, 128], bf16, tag=f"ds{qc}", name=f"ds{qc}")
                       for qc in range(2)]

            # ---- DMA issue order: tiny, wk, xT, wq, wv, mask, wo ----
            nc.sync.dma_start(ones_r[:], ones_rd[:])
            ab_cm = tc.tile_pool(name="attn", bufs=1)
            ab = ab_cm.__enter__()
            exs_cm = tc.tile_pool(name="exs", bufs=3)
            exs = exs_cm.__enter__()
            wts = {}
            xtp = tc.tile_pool(name="xw", bufs=1)
            xw = xtp.__enter__()
            xt = [xw.tile([128, S], bf16, tag=f"x{d}", name=f"x{d}") for d in range(8)]
            for nm in ("wq", "wk", "wv"):
                wts[nm] = [xw.tile([128, DH], bf16, tag=f"{nm}{d}", name=f"{nm}{d}") for d in range(8)]
            for d in range(8):
                nc.sync.dma_start(wts["wk"][d][:], wk[d * 128:(d + 1) * 128, :])
            for d in range(8):
                nc.sync.dma_start(xt[d][:], xT[d * 128:(d + 1) * 128, :])
            for d in range(8):
                nc.sync.dma_start(wts["wq"][d][:], wq[d * 128:(d + 1) * 128, :])
            for d in range(8):
                nc.sync.dma_start(wts["wv"][d][:], wv[d * 128:(d + 1) * 128, :])
            imaskT_r = imaskT.rearrange("(t p) q -> p t q", p=128)
            for kb in range(16):
                nc.sync.dma_start(mt[:, kb:kb + 1, :], imaskT_r[:, kb:kb + 1, :])
            for g in range(2):
                nc.sync.dma_start(wot[g][:], wo[g * 128:(g + 1) * 128, :])
            for st in range(16):
                nc.sync.dma_start(vt[st][:, :, DEPTH:DEPTH + 1], vones[:])

            # identity (bf16) + exp-table preload + PE warm spin
            make_identity(nc, identb[:])
            nc.scalar.activation(dummy[:], identb[0:1, 0:64], EXP, scale=1.0)
            with tc.tile_pool(name="psW", bufs=2, space="PSUM") as psW:
                for w in range(28):
                    psw = psW.tile([128, 128], f32, tag="warm", name="warm")
                    nc.tensor.matmul(psw[:], identb[:], identb[:],
                                     start=True, stop=True)

            # ---- projections (k, then q chunks sc0/sc1, then v) ----
            psA_cm = tc.tile_pool(name="psA", bufs=4, space="PSUM")
            psA = psA_cm.__enter__()

            def proj_qk(wt, dst, g, sc, pool, tag="proj"):
                ps = pool.tile([128, 512], f32, tag=tag, name=tag)
                for d in range(8):
                    nc.tensor.matmul(
                        ps[:],
                        wt[d][:, g * 128:(g + 1) * 128],
                        xt[d][:, sc * 512:(sc + 1) * 512],
                        start=(d == 0), stop=(d == 7),
                    )
                nc.vector.tensor_copy(dst[g][:, sc * 512:(sc + 1) * 512], ps[:])

            for g in range(2):
                for sc in range(4):
                    proj_qk(wts["wk"], kT, g, sc, psA)
            for g in range(2):
                for sc in range(2):
                    proj_qk(wts["wq"], qT, g, sc, psA)
            for st in range(16):
                ps = psA.tile([128, DH], f32, tag="proj", name="proj")
                for d in range(8):
                    nc.tensor.matmul(
                        ps[:],
                        xt[d][:, st * 128:(st + 1) * 128],
                        wts["wv"][d][:],
                        start=(d == 0), stop=(d == 7),
                    )
                nc.vector.tensor_copy(
                    vt[st][:, :, 0:DEPTH],
                    ps[:].rearrange("p (h e) -> p h e", h=HPC),
                )
            psA_cm.__exit__(None, None, None)

            # ---- attention ----
            with tc.tile_pool(name="psL", bufs=2, space="PSUM") as psL, \
                 tc.tile_pool(name="psO", bufs=1, space="PSUM") as psO, \
                 tc.tile_pool(name="psB", bufs=1, space="PSUM") as psB:

                def emit_qk(qcp, h, kb):
                    g, po = h // 2, (h % 2) * 64
                    psl = psL.tile([128, 1024], f32, tag="lg", name="lg")
                    for half in range(2):
                        hs = slice(half * 512, (half + 1) * 512)
                        qh = slice(qcp * 1024 + half * 512,
                                   qcp * 1024 + half * 512 + 512)
                        nc.tensor.matmul(
                            psl[:, hs],
                            kT[g][po:po + 64, kb * 128:(kb + 1) * 128],
                            qT[g][po:po + 64, qh],
                            start=True, stop=True,
                        )
                    return psl

                # deferred PE side-tasks, popped one per inner unit
                pe_tasks = []

                def emit_q23():
                    for g in range(2):
                        for sc in range(2, 4):
                            pe_tasks.append(
                                lambda g=g, sc=sc: proj_qk(
                                    wts["wq"], qT, g, sc, psB, tag="po"))

                def emit_outproj(qcp):
                    # 8 st tiles x (2 nch psum [128,512] x 2 g) + copies + DMA
                    def one_st(st):
                        ot = ab.tile([128, D], bf16, tag="ot", name="ot", bufs=2)
                        for nch in range(2):
                            psf = psB.tile([128, 512], f32, tag="po", name="po")
                            for g in range(2):
                                nc.tensor.matmul(
                                    psf[:],
                                    ath[qcp][g][:, st * 128 - qcp * 1024:
                                                st * 128 - qcp * 1024 + 128],
                                    wot[g][:, nch * 512:(nch + 1) * 512],
                                    start=(g == 0), stop=(g == 1),
                                )
                            if nch == 0:
                                nc.scalar.copy(
                                    ot[:, nch * 512:(nch + 1) * 512], psf[:])
                            else:
                                nc.vector.tensor_copy(
                                    ot[:, nch * 512:(nch + 1) * 512], psf[:])
                        nc.sync.dma_start(out[st * 128:(st + 1) * 128, :], ot[:])
                    for st in range(qcp * 8, qcp * 8 + 8):
                        pe_tasks.append(lambda st=st: one_st(st))

                def head_den_dma(qcp, h):
                    # park this head's denominator row in DRAM scratch
                    nc.sync.dma_start(den_scr[qcp][h * 8:(h + 1) * 8, :],
                                      dden[qcp][0:1, h, :])

                def pair_gather(qcp, g):
                    # XBAR-transpose both heads' dens to partition-major and
                    # take one [128,16] reciprocal.
                    sl = slice(g * 16, (g + 1) * 16)
                    nc.sync.dma_start_transpose(den_pm[qcp][:, sl],
                                                den_scr[qcp][sl, :])
                    with nc.allow_low_precision(reason="bf16 denominators"):
                        nc.vector.reciprocal(rden_pm[qcp][:, sl],
                                             den_pm[qcp][:, sl])

                def pair_norm(qcp, g):
                    # tiny PE transposes back to a row, rank-1 broadcast into
                    # psum, then scale both heads' attn rows in place.
                    for hh in range(2):
                        h = 2 * g + hh
                        for half in range(2):
                            psb = psB.tile([128, 512], bf16, tag="dt",
                                           name="dt")
                            for c in range(4):
                                cc = h * 8 + half * 4 + c
                                nc.tensor.transpose(
                                    psb[0:1, c * 128:(c + 1) * 128],
                                    rden_pm[qcp][:, cc:cc + 1],
                                    identb[:],
                                )
                            nc.scalar.copy(
                                rrow[qcp][0:1, h * 1024 + half * 512:
                                          h * 1024 + half * 512 + 512],
                                psb[0:1, 0:512])
                    for half in range(2):
                        hs = slice(half * 512, (half + 1) * 512)
                        psc = psB.tile([128, 512], f32, tag="po", name="psc")
                        for hh in range(2):
                            h = 2 * g + hh
                            nc.tensor.matmul(
                                psc[hh * 64:(hh + 1) * 64, :],
                                ones_r[:],
                                rrow[qcp][0:1, h * 1024 + half * 512:
                                          h * 1024 + half * 512 + 512],
                                start=True, stop=True,
                            )
                        nc.vector.tensor_tensor(
                            ath[qcp][g][:, hs],
                            ath[qcp][g][:, hs], psc[:], MULT)

                pend_norms = []  # (due_global_unit, qcp, g)
                gu = 0
                for qcp in range(2):
                    if qcp == 0:
                        emit_q23()
                    else:
                        xtp.__exit__(None, None, None)
                        emit_outproj(0)
                    units = [(h, kb) for h in range(HPC) for kb in range(16)]
                    psl_next = emit_qk(qcp, 0, 0)
                    for i, (h, kb) in enumerate(units):
                        g, po = h // 2, (h % 2) * 64
                        psl = psl_next
                        if i + 1 < len(units):
                            hn, kbn = units[i + 1]
                            psl_next = emit_qk(qcp, hn, kbn)
                        ex = exs.tile([128, 1024], bf16, tag="ex", name="ex", bufs=4)
                        if (kb % 16) in DVE_KBS:
                            nc.vector.tensor_scalar(
                                ex[:].bitcast(u16), psl[:],
                                SCHR_A, SCHR_B, MULT, ADD)
                        else:
                            nc.scalar.activation(ex[:], psl[:], EXP, scale=0.125)
                        em = exs.tile([128, 1024], bf16, tag="em", name="em", bufs=4)
                        nc.vector.tensor_tensor(
                            em[:], ex[:], mt[:, kb, qcp * 1024:(qcp + 1) * 1024],
                            MULT)
                        for half in range(2):
                            hs = slice(half * 512, (half + 1) * 512)
                            pso = psO.tile([65, 512], f32, tag=f"av{half}",
                                           name=f"av{half}")
                            nc.tensor.matmul(
                                pso[:], vt[kb][:, h, :], em[:, hs],
                                start=(kb == 0), stop=(kb == 15),
                            )
                            if kb == 15:
                                nc.vector.tensor_copy(
                                    ath[qcp][g][po:po + 64, hs], pso[0:64, :])
                                nc.scalar.copy(
                                    dden[qcp][0:1, h, hs], pso[64:65, :])
                        if kb == 15:
                            head_den_dma(qcp, h)
                            if h % 2 == 1:
                                pair_gather(qcp, g)
                                pend_norms.append((gu + 4, qcp, g))
                        if pend_norms and pend_norms[0][0] <= gu:
                            _, pq, pg = pend_norms.pop(0)
                            pair_norm(pq, pg)
                        if pe_tasks and ((qcp == 0 and i % 4 == 1) or
                                         (qcp == 1 and i >= 5 and i % 2 == 1)):
                            pe_tasks.pop(0)()
                        gu += 1
                    while qcp == 0 and pe_tasks:
                        pe_tasks.pop(0)()
                while pend_norms:
                    _, pq, pg = pend_norms.pop(0)
                    pair_norm(pq, pg)
                emit_outproj(1)
                while pe_tasks:
                    pe_tasks.pop(0)()
            exs_cm.__exit__(None, None, None)
            ab_cm.__exit__(None, None, None)
            dsc_cm.__exit__(None, None, None)

    nc.compile()
    nc.m = get_hw_module(nc.m)
    return nc


def _get_program():
    if "nc" not in _CACHE:
        _CACHE["nc"] = _build_program()
    return _CACHE["nc"]


def _make_in_maps(query, attention_mask, Wq, Wk, Wv, Wo):
    import ml_dtypes

    bf = ml_dtypes.bfloat16
    in_maps = []
    imaskT_b = []
    xT_b = []
    for b in range(B):
        imaskT_b.append(
            np.ascontiguousarray(1 - attention_mask[b, 0].T).astype(bf)
        )
        xT_b.append(np.ascontiguousarray(query[b].T).astype(bf))
    for c in range(CORES):
        b, hg = c // HG, c % HG
        cs = slice(hg * DH, (hg + 1) * DH)
        in_maps.append({
            "xT": xT_b[b],
            "imaskT": imaskT_b[b],
            "wq": np.ascontiguousarray(Wq[:, cs]).astype(bf),
            "wk": np.ascontiguousarray(Wk[:, cs]).astype(bf),
            "wv": np.ascontiguousarray(Wv[:, cs]).astype(bf),
            "wo": np.ascontiguousarray(Wo[cs, :]).astype(bf),
            "vones": np.ones((128, HPC, 1), dtype=bf),
            "ones_rd": np.ones((1, DEPTH), dtype=bf),
        })
    return in_maps


def _run(inputs, trace=False):
    from concourse.bass_utils import run_bass_kernel_spmd

    nc = _get_program()
    in_maps = _make_in_maps(**inputs)
    res = run_bass_kernel_spmd(
        nc, in_maps, core_ids=list(range(CORES)), trace=trace,
    )
    outs = [res.results[c]["out"].astype(np.float32) for c in range(CORES)]
    full = np.empty((B, S, D), dtype=np.float32)
    for b in range(B):
        acc = outs[4 * b]
        for hg in range(1, HG):
            acc = acc + outs[4 * b + hg]
        full[b] = acc
    return full, res


def kernel(query, attention_mask, Wq, Wk, Wv, Wo):
    full, _ = _run(dict(
        query=np.asarray(query), attention_mask=np.asarray(attention_mask),
        Wq=np.asarray(Wq), Wk=np.asarray(Wk), Wv=np.asarray(Wv),
        Wo=np.asarray(Wo),
    ))
    return full
